# revision 7
# baseline (speedup 1.0000x reference)
"""Raw-bass variant (no TileContext): same GEMM dataflow as the
checkpoint, but with hand-placed semaphores so each engine's program
ends immediately after its last real instruction -- the goal is to skip
the tile scheduler's ~1.7us end-of-kernel drain/barrier cascade.

sync:   dma(blob<-xw) +16 | wait cp | dma(oa<-ot) +16 | wait out | clear sems
tensor: wait in | matmul +1
vector: wait mm | f16 cast +1   (DVE: 423ns vs ACT 473ns for 256 cols)
(The trailing sem_clears make repeat executions correct: they run after
every other engine's waits have already consumed the sems.  The
constructor preamble's const-AP memsets and all-engine barrier are
deleted before compile -- manual sems fully order the program.)
"""

import os

import numpy as np

N = 256      # neurons
I = 784      # input dim
H = 128      # hidden dim
B = 256      # batch
M_CORES = 8
ISL = I // M_CORES          # 98 contraction rows per core

_compiled = None
_last_results = None  # for test harness introspection


def _build():
    import concourse.bacc as bacc
    import concourse.mybir as mybir

    nc = bacc.Bacc(
        "TRN2",
        target_bir_lowering=False,
        debug=False,
        num_devices=M_CORES,
    )
    f32 = mybir.dt.float32
    f16 = mybir.dt.float16

    entry = nc.main_func.blocks[0]
    n_pre = len(entry.instructions)

    xw = nc.dram_tensor("xw", [ISL, B + H], f16, kind="ExternalInput")
    oa = nc.dram_tensor("oa", [H, B], f16, kind="ExternalOutput")

    s_in = nc.alloc_semaphore("s_in")
    s_mm = nc.alloc_semaphore("s_mm")
    s_cp = nc.alloc_semaphore("s_cp")
    s_out = nc.alloc_semaphore("s_out")

    blob = nc.alloc_sbuf_tensor("blob", [ISL, B + H], f16)
    ot = nc.alloc_sbuf_tensor("ot", [H, B], f16)
    ps = nc.alloc_psum_tensor("ps", [H, B], f32)

    nc.sync.dma_start(blob.ap(), xw[:]).then_inc(s_in, 16)

    nc.tensor.wait_ge(s_in, 16)
    nc.tensor.matmul(
        ps.ap(), blob.ap()[:, B : B + H], blob.ap()[:, 0:B],
        start=True, stop=True,
    ).then_inc(s_mm, 1)

    nc.vector.wait_ge(s_mm, 1)
    nc.vector.tensor_scalar_mul(ot.ap(), ps.ap(), 1.0).then_inc(s_cp, 1)

    nc.sync.wait_ge(s_cp, 1)
    nc.sync.dma_start(oa[:], ot.ap()).then_inc(s_out, 16)
    # wait for the store's write receipt: this anchors the profiler's
    # exec window at the true end of the kernel's work (without it the
    # reported time excludes the in-flight store)
    nc.sync.wait_ge(s_out, 16)
    nc.sync.sem_clear(s_in)
    nc.sync.sem_clear(s_mm)
    nc.sync.sem_clear(s_cp)
    nc.sync.sem_clear(s_out)

    # drop the constructor preamble's const-AP memsets and all-engine
    # barrier (drains + barrier_* sems): no const AP is read and the
    # manual semaphores fully order the program, so engines can enter
    # user code ~0.7us earlier
    drop = [
        ins for ins in entry.instructions[:n_pre]
        if type(ins).__name__ in ("InstMemset", "InstDrain")
        or str(getattr(ins, "name", "")).startswith("barrier_")
    ]
    for ins in drop:
        entry.instructions.remove(ins)

    nc.compile()
    return nc


def _compute_v(edge_index, edge_weights):
    src = np.asarray(edge_index[0], dtype=np.int64)
    tgt = np.asarray(edge_index[1], dtype=np.int64)
    ew = np.asarray(edge_weights, dtype=np.float64)
    u = np.ones(N, dtype=np.float64)
    for e in range(ew.shape[0] - 1, -1, -1):
        u[src[e]] += ew[e] * u[tgt[e]]
    return (u / N).astype(np.float32)


def kernel(x, W, b, edge_index, edge_weights):
    global _compiled, _last_results
    from concourse.bass_utils import run_bass_kernel_spmd

    x = np.asarray(x, dtype=np.float32)
    W = np.asarray(W, dtype=np.float32)
    b = np.asarray(b, dtype=np.float32)

    v = _compute_v(edge_index, edge_weights)
    b_eff = (v.astype(np.float64) @ b.astype(np.float64)).astype(np.float32)
    w_eff = np.tensordot(v, W, axes=1)          # (I, H) f32
    xT = np.ascontiguousarray(x.T)              # (I, B) f32

    if _compiled is None:
        _compiled = _build()

    in_maps = []
    for c in range(M_CORES):
        rows = slice(c * ISL, (c + 1) * ISL)
        blob = np.concatenate([xT[rows], w_eff[rows]], axis=1)
        in_maps.append({"xw": blob.astype(np.float16)})

    trace = bool(int(os.environ.get("KERNEL_TRACE", "0")))
    res = run_bass_kernel_spmd(
        _compiled, in_maps, core_ids=list(range(M_CORES)), trace=trace
    )
    _last_results = res

    acc = np.zeros((H, B), dtype=np.float32)
    for r in res.results:
        acc += r["oa"].astype(np.float32)
    return (acc.T + b_eff[None, :]).astype(np.float32)
